# revision 45
# baseline (speedup 1.0000x reference)
"""ATSP encoder (5-layer dual-stream AFT transformer) on 8 TRN2 NeuronCores.

Sharding: data-parallel over batch B=128 -> 16 items per core, params
replicated. Per core the whole network runs out of SBUF per batch item.

Layout: residual streams are kept transposed [D(part), seq(free)] so that
instance-norm (reduce over seq) is a free-axis bn_stats, the per-channel
affine is per-partition, and FF/projection matmuls contract naturally.
k/v are produced in [seq, D] (activation as matmul lhsT), and the AFT GEMMs
compute numT/denT = lhsT(ekv|ek).T @ rhs(E^T) straight back into the
transposed layout -- no transposes anywhere in the layer loop.

Matmuls run in bf16 (fp32 PSUM accumulation); the residual stream stays
fp32. alpha/log_scale are folded into compile-time exp() scales and b2 is
dropped (a per-channel constant shift cancels exactly in instance norm).
"""

import numpy as np

B, NSEQ, D, F, L = 128, 512, 256, 512, 5
NCORES = 8
BLOC = B // NCORES
P = 128
DCH, SCH, FCH = D // P, NSEQ // P, F // P
EPS = 1e-5

_CACHE: dict = {}
LAST_RESULT = None


def _build(scales_r, scales_c, bloc=BLOC, enable_asserts=False, num_devices=NCORES,
           unit_g1=False, zero_be1=False, unit_g2=False, zero_be2=False, zero_b1=False):
    from contextlib import ExitStack

    import concourse.bacc as bacc
    import concourse.mybir as mybir
    import concourse.tile as tile
    from concourse.masks import make_identity

    dt = mybir.dt
    AF = mybir.ActivationFunctionType
    OP = mybir.AluOpType
    f32 = dt.float32
    bf16 = dt.bfloat16

    nc = bacc.Bacc(
        "TRN2",
        target_bir_lowering=False,
        debug=False,
        enable_asserts=enable_asserts,
        num_devices=num_devices,
    )

    row_d = nc.dram_tensor("row_emb", [bloc, NSEQ, D], f32, kind="ExternalInput").ap()
    col_d = nc.dram_tensor("col_emb", [bloc, NSEQ, D], f32, kind="ExternalInput").ap()
    cost_d = nc.dram_tensor("cost_mat", [bloc, NSEQ, NSEQ], f32, kind="ExternalInput").ap()
    wq_d = nc.dram_tensor("Wq", [L, 2, D, D], f32, kind="ExternalInput").ap()
    wk_d = nc.dram_tensor("Wk", [L, 2, D, D], f32, kind="ExternalInput").ap()
    wv_d = nc.dram_tensor("Wv", [L, 2, D, D], f32, kind="ExternalInput").ap()
    g1_d = nc.dram_tensor("g1", [L, 2, D], f32, kind="ExternalInput").ap()
    be1_d = nc.dram_tensor("be1", [L, 2, D], f32, kind="ExternalInput").ap()
    w1_d = nc.dram_tensor("W1", [L, 2, D, F], f32, kind="ExternalInput").ap()
    b1_d = nc.dram_tensor("b1", [L, 2, F], f32, kind="ExternalInput").ap()
    w2_d = nc.dram_tensor("W2", [L, 2, F, D], f32, kind="ExternalInput").ap()
    g2_d = nc.dram_tensor("g2", [L, 2, D], f32, kind="ExternalInput").ap()
    be2_d = nc.dram_tensor("be2", [L, 2, D], f32, kind="ExternalInput").ap()
    out_d = nc.dram_tensor("out", [2, bloc, NSEQ, D], f32, kind="ExternalOutput").ap()

    with tile.TileContext(nc) as tc, ExitStack() as ctx:
        # Pre-load the combined exp+ln activation table set once. Every
        # activation in this kernel (Exp, Ln, Relu, Copy, Identity) lives in
        # this one set, so the fixpoint table-load pass inserts no further
        # ACT_TABLE_LOADs (saves ~4 reloads x ~1.3us per layer-side).
        from concourse.hw_specs import get_activation_tables

        table_names = list(get_activation_tables(nc.m.arch))
        combined_id = table_names.index("natural_log_exp_and_others")
        nc.scalar.add_instruction(
            mybir.InstLoadActFuncSet(
                act_func_set_id=combined_id,
                name=nc.get_next_instruction_name(),
                ins=[],
                outs=[],
            )
        )

        consts = ctx.enter_context(tc.tile_pool(name="consts", bufs=1))
        wpool = ctx.enter_context(tc.tile_pool(name="wpool", bufs=1))

        ident = consts.tile([P, P], f32)
        make_identity(nc, ident)
        epsc = consts.tile([P, 1], f32)
        nc.vector.memset(epsc, EPS)

        g1a = consts.tile([P, L * 2 * DCH], f32)
        nc.sync.dma_start(g1a, g1_d.rearrange("l s (c ci) -> ci (l s c)", ci=P))
        be1a = consts.tile([P, L * 2 * DCH], f32)
        nc.sync.dma_start(be1a, be1_d.rearrange("l s (c ci) -> ci (l s c)", ci=P))
        g2a = consts.tile([P, L * 2 * DCH], f32)
        nc.sync.dma_start(g2a, g2_d.rearrange("l s (c ci) -> ci (l s c)", ci=P))
        be2a = consts.tile([P, L * 2 * DCH], f32)
        nc.sync.dma_start(be2a, be2_d.rearrange("l s (c ci) -> ci (l s c)", ci=P))
        b1a = consts.tile([P, L * 2 * FCH], f32)
        nc.sync.dma_start(b1a, b1_d.rearrange("l s (c ci) -> ci (l s c)", ci=P))

        with tc.tile_pool(name="wstage", bufs=2) as wstage:

            def load_w(dram_ap, ko_cnt, o_dim, name):
                stgt = wstage.tile([P, L * 2 * ko_cnt, o_dim], f32, tag="wstg", name=f"stg_{name}")
                nc.sync.dma_start(
                    stgt, dram_ap.rearrange("l s (ko ki) o -> ki (l s ko) o", ki=P)
                )
                wb = wpool.tile([P, L * 2 * ko_cnt, o_dim], bf16, name=name)
                nc.vector.tensor_copy(wb, stgt)
                return wb

            WqB = load_w(wq_d, DCH, D, "WqB")
            W1B = load_w(w1_d, DCH, F, "W1B")
            W2B = load_w(w2_d, FCH, D, "W2B")
            # pack [Wk | Wv] along the output dim: one N=512 rhs for the
            # k|v matmuls (half the matmul/LDWEIGHTS instruction count)
            WkvB = wpool.tile([P, L * 2 * DCH, 2 * D], bf16, name="WkvB")
            for w_d, off in ((wk_d, 0), (wv_d, D)):
                stgt = wstage.tile(
                    [P, L * 2 * DCH, D], f32, tag="wstg", name=f"stg_kv{off}"
                )
                nc.sync.dma_start(
                    stgt, w_d.rearrange("l s (ko ki) o -> ki (l s ko) o", ki=P)
                )
                nc.vector.tensor_copy(WkvB[:, :, off : off + D], stgt)

        cmp_ = ctx.enter_context(tc.tile_pool(name="cmpool", bufs=1))
        epool = ctx.enter_context(tc.tile_pool(name="epool", bufs=2))
        stg = ctx.enter_context(tc.tile_pool(name="stgpool", bufs=2))
        opool = ctx.enter_context(tc.tile_pool(name="opool", bufs=1))
        ipool = ctx.enter_context(tc.tile_pool(name="ipool", bufs=1))
        kvp = ctx.enter_context(tc.tile_pool(name="kvp", bufs=4))
        xp = ctx.enter_context(tc.tile_pool(name="xp", bufs=3))
        strm = ctx.enter_context(tc.tile_pool(name="strm", bufs=2))
        tpool = ctx.enter_context(tc.tile_pool(name="tpool", bufs=2))
        ttp = ctx.enter_context(tc.tile_pool(name="ttp", bufs=3))
        spool = ctx.enter_context(tc.tile_pool(name="spool", bufs=4))
        psp = ctx.enter_context(tc.tile_pool(name="psp", bufs=8, space="PSUM"))

        INV_N = 1.0 / NSEQ

        def in_norm(xin, xsum, ga, bea, unit_g, zero_be, lsi, outf, outb):
            """xin [P, DCH, NSEQ], xsum [P, DCH] = per-partition sums of xin
            (accumulated by the producing adds). Variance via E[x^2]-mu^2:
            the sum of squares comes from an ACT Square pass with accum_out,
            keeping the whole stats chain off the DVE critical path."""
            qsum = spool.tile([P, DCH], f32, tag="qsum", name="qsum")
            for do in range(DCH):
                ssq = ttp.tile([P, NSEQ], f32, tag="tt", name="ssq")
                if do == 0:
                    nc.scalar.activation(
                        ssq, xin[:, do, :], AF.Square,
                        accum_out=qsum[:, do : do + 1],
                    )
                else:
                    # second chunk's sum-of-squares on DVE, in parallel with
                    # the ACT square of the first chunk
                    nc.vector.scalar_tensor_tensor(
                        ssq, xin[:, do, :], 0.0, xin[:, do, :], OP.add, OP.mult,
                        accum_out=qsum[:, do : do + 1],
                    )
            # b = sumsq - xsum^2/N  (=> var = b/N)
            a = spool.tile([P, DCH], f32, tag="a", name="a")
            bvar = spool.tile([P, DCH], f32, tag="bvar", name="bvar")
            with tc.high_priority(offset=24):
                nc.vector.scalar_tensor_tensor(a, xsum, INV_N, xsum, OP.mult, OP.mult)
                nc.vector.tensor_sub(bvar, qsum, a)
            # 1/sqrt(var+eps) = exp(-0.5*ln(b/N+eps)): stays in the exp/ln
            # ACT table set, so the whole kernel runs on one resident table.
            lnv = spool.tile([P, DCH], f32, tag="lnv", name="lnv")
            nc.scalar.activation(lnv, bvar, AF.Ln, bias=epsc, scale=INV_N)
            rs = spool.tile([P, DCH], f32, tag="rs", name="rs")
            nc.scalar.activation(rs, lnv, AF.Exp, scale=-0.5)
            if unit_g:
                s1 = rs
            else:
                s1 = spool.tile([P, DCH], f32, tag="s1", name="s1")
                nc.vector.tensor_mul(s1, rs, ga[:, lsi * DCH : (lsi + 1) * DCH])
            bb = spool.tile([P, DCH], f32, tag="bb", name="bb")
            if zero_be:
                # bb = -mean * s1 = -(xsum/N) * s1
                with tc.high_priority(offset=24):
                    nc.vector.scalar_tensor_tensor(bb, xsum, -INV_N, s1, OP.mult, OP.mult)
            else:
                ms = spool.tile([P, DCH], f32, tag="ms", name="ms")
                nc.vector.scalar_tensor_tensor(ms, xsum, -INV_N, s1, OP.mult, OP.mult)
                nc.vector.tensor_add(bb, bea[:, lsi * DCH : (lsi + 1) * DCH], ms)
            # split the applies across DVE and GPSIMD so the two chunks run
            # in parallel instead of serializing on one engine
            napply = 0
            for do in range(DCH):
                for out in (outf, outb):
                    if out is None:
                        continue
                    eng = nc.vector if napply % 2 == 0 else nc.gpsimd
                    eng.tensor_scalar(
                        out[:, do, :], xin[:, do, :],
                        s1[:, do : do + 1], bb[:, do : do + 1],
                        OP.mult, OP.add,
                    )
                    napply += 1

        def q_stage(lsi, xTb):
            # q -> eq = exp(-q) in [D, n]; sigmoid(q)*num/den is computed as
            # num / (den * (1+eq)), so no sigmoid table set is ever needed.
            u = kvp.tile([P, DCH, NSEQ], bf16, tag="u", name="u")
            for mo in range(DCH):
                qps = psp.tile([P, NSEQ], f32, tag="ps", name=f"qps{lsi}_{mo}")
                for ko in range(DCH):
                    nc.tensor.matmul(
                        qps,
                        WqB[:, lsi * DCH + ko, mo * P : (mo + 1) * P],
                        xTb[:, ko, :],
                        start=(ko == 0), stop=(ko == DCH - 1),
                    )
                nc.scalar.activation(u[:, mo, :], qps, AF.Exp, scale=-1.0)
            return u

        def kv_stage(lsi, yTb):
            # k|v packed per seq-chunk, in [m, D]
            ek = kvp.tile([P, SCH, D], bf16, tag="ek", name="ek")
            ekv = kvp.tile([P, SCH, D], bf16, tag="ekv", name="ekv")
            for sc in range(SCH):
                kvps = psp.tile([P, NSEQ], f32, tag="ps", name=f"kvps{lsi}_{sc}")
                for ko in range(DCH):
                    nc.tensor.matmul(
                        kvps,
                        yTb[:, ko, sc * P : (sc + 1) * P],
                        WkvB[:, lsi * DCH + ko, :],
                        start=(ko == 0), stop=(ko == DCH - 1),
                    )
                with tc.high_priority(offset=16):
                    nc.scalar.activation(ek[:, sc, :], kvps[:, 0:D], AF.Exp)
                    nc.vector.tensor_mul(ekv[:, sc, :], kvps[:, D : 2 * D], ek[:, sc, :])
            return ek, ekv

        def aft_stage(lsi, u, ek, ekv, E, xT):
            # AFT: numT/denT [D, n] = (ekv|ek).T @ E^T, then combine + residual
            x1 = xp.tile([P, DCH, NSEQ], f32, tag="x1", name="x1")
            x1sum = spool.tile([P, DCH], f32, tag="xsum", name="x1sum")
            for do in range(DCH):
                # den first: the (1+eq)*den + reciprocal chain runs on DVE
                # while the num matmuls stream on PE
                dps = psp.tile([P, NSEQ], f32, tag="ps", name=f"dps{lsi}_{do}")
                for sc in range(SCH):
                    nc.tensor.matmul(
                        dps, ek[:, sc, do * P : (do + 1) * P], E[:, sc, :],
                        start=(sc == 0), stop=(sc == SCH - 1),
                    )
                dd = ttp.tile([P, NSEQ], f32, tag="tt", name="dd")
                # dd = (eq + 1) * den  -- folds the sigmoid denominator in
                with tc.high_priority(offset=16):
                    nc.vector.scalar_tensor_tensor(dd, u[:, do, :], 1.0, dps, OP.add, OP.mult)
                rdd = ttp.tile([P, NSEQ], f32, tag="tt", name="rdd")
                nc.vector.reciprocal_approx_fast(rdd, dd)
                nps = psp.tile([P, NSEQ], f32, tag="ps", name=f"nps{lsi}_{do}")
                for sc in range(SCH):
                    nc.tensor.matmul(
                        nps, ekv[:, sc, do * P : (do + 1) * P], E[:, sc, :],
                        start=(sc == 0), stop=(sc == SCH - 1),
                    )
                t = ttp.tile([P, NSEQ], f32, tag="tt", name="t")
                with tc.high_priority(offset=16):
                    nc.vector.tensor_mul(t, nps, rdd)
                nc.vector.scalar_tensor_tensor(
                    x1[:, do, :], t, 0.0, xT[:, do, :], OP.add, OP.add,
                    accum_out=x1sum[:, do : do + 1],
                )
            return x1, x1sum

        def in1_stage(lsi, x1, x1sum):
            # h1 is kept only in bf16: it feeds the ff1 matmuls directly and
            # the ff2 residual add (bf16 rounding there is within budget)
            h1b = tpool.tile([P, DCH, NSEQ], bf16, tag="h1b", name="h1b")
            in_norm(x1, x1sum, g1a, be1a, unit_g1, zero_be1, lsi, None, h1b)
            return h1b

        def ff1_stage(lsi, h1b):
            ff1b = tpool.tile([P, FCH, NSEQ], bf16, tag="ff1b", name="ff1b")
            for fo in range(FCH):
                fps = psp.tile([P, NSEQ], f32, tag="ps", name=f"fps{lsi}_{fo}")
                for ko in range(DCH):
                    nc.tensor.matmul(
                        fps,
                        W1B[:, lsi * DCH + ko, fo * P : (fo + 1) * P],
                        h1b[:, ko, :],
                        start=(ko == 0), stop=(ko == DCH - 1),
                    )
                if zero_b1:
                    nc.scalar.activation(ff1b[:, fo, :], fps, AF.Relu, bias=0.0)
                else:
                    nc.scalar.activation(
                        ff1b[:, fo, :], fps, AF.Relu,
                        bias=b1a[:, lsi * FCH + fo : lsi * FCH + fo + 1],
                    )
            return ff1b

        def ff2_stage(lsi, ff1b, h1b):
            x2 = xp.tile([P, DCH, NSEQ], f32, tag="x1", name="x2")
            x2sum = spool.tile([P, DCH], f32, tag="xsum", name="x2sum")
            for do in range(DCH):
                f2ps = psp.tile([P, NSEQ], f32, tag="ps", name=f"f2ps{lsi}_{do}")
                for ko in range(FCH):
                    nc.tensor.matmul(
                        f2ps,
                        W2B[:, lsi * FCH + ko, do * P : (do + 1) * P],
                        ff1b[:, ko, :],
                        start=(ko == 0), stop=(ko == FCH - 1),
                    )
                nc.vector.scalar_tensor_tensor(
                    x2[:, do, :], f2ps, 0.0, h1b[:, do, :], OP.add, OP.add,
                    accum_out=x2sum[:, do : do + 1],
                )
            return x2, x2sum

        def in2_stage(lsi, x2, x2sum, last):
            s = lsi % 2
            nx = strm.tile([P, DCH, NSEQ], f32, tag=f"x{s}", name=f"x{s}")
            nxb = None
            if not last:
                nxb = strm.tile([P, DCH, NSEQ], bf16, tag=f"xb{s}", name=f"xb{s}")
            in_norm(x2, x2sum, g2a, be2a, unit_g2, zero_be2, lsi, nx, nxb)
            return nx, nxb

        def enc_pair(l, xs, Er, Ec, last):
            # Interleave the two independent sides of a layer, with the col
            # side staggered ~1.5 stages behind the row side so the two
            # serial norm chains never coincide -- one side's matmuls keep
            # PE fed while the other side's stats/apply chain runs.
            lsr, lsc = l * 2, l * 2 + 1
            (xr, xrb), (xc, xcb) = xs[0], xs[1]
            ur = q_stage(lsr, xrb)
            ekr, ekvr = kv_stage(lsr, xcb)
            uc = q_stage(lsc, xcb)
            x1r, x1sr = aft_stage(lsr, ur, ekr, ekvr, Er, xr)
            ekc, ekvc = kv_stage(lsc, xrb)
            h1br = in1_stage(lsr, x1r, x1sr)
            x1c, x1sc = aft_stage(lsc, uc, ekc, ekvc, Ec, xc)
            f1r = ff1_stage(lsr, h1br)
            h1bc = in1_stage(lsc, x1c, x1sc)
            x2r, x2sr = ff2_stage(lsr, f1r, h1br)
            f1c = ff1_stage(lsc, h1bc)
            nr = in2_stage(lsr, x2r, x2sr, last)
            x2c, x2sc = ff2_stage(lsc, f1c, h1bc)
            ncl = in2_stage(lsc, x2c, x2sc, last)
            return nr, ncl

        for b in range(bloc):
            cm = cmp_.tile([P, SCH, NSEQ], f32, tag="cm", name="cm")
            nc.sync.dma_start(cm, cost_d[b].rearrange("(no ni) m -> ni no m", ni=P))

            def get_Ec(scale, b=b):
                Ec = epool.tile([P, SCH, NSEQ], bf16, tag="Ec", name="Ec")
                for no in range(SCH):
                    nc.scalar.activation(Ec[:, no, :], cm[:, no, :], AF.Exp, scale=scale)
                return Ec

            def get_Er(scale, b=b):
                Er = epool.tile([P, SCH, NSEQ], bf16, tag="Er", name="Er")
                for mo in range(SCH):
                    pt = psp.tile([P, NSEQ], f32, tag="ps", name=f"ept{b}_{mo}")
                    for no in range(SCH):
                        nc.tensor.transpose(
                            pt[:, no * P : (no + 1) * P],
                            cm[:, no, mo * P : (mo + 1) * P],
                            ident,
                        )
                    nc.scalar.activation(Er[:, mo, :], pt, AF.Exp, scale=scale)
                return Er

            xs = {}
            for s, src in ((0, row_d), (1, col_d)):
                xnd = stg.tile([P, SCH, D], f32, tag="xnd", name="xnd")
                nc.sync.dma_start(xnd, src[b].rearrange("(no ni) d -> ni no d", ni=P))
                # dedicated tags for the item-initial stream tiles so the
                # next item's input staging never rotates against this
                # item's per-layer stream tiles (cross-item overlap)
                xT = ipool.tile([P, DCH, NSEQ], f32, tag=f"xi{s}", name=f"xi{s}")
                xTb = ipool.tile([P, DCH, NSEQ], bf16, tag=f"xbi{s}", name=f"xbi{s}")
                for do in range(DCH):
                    pt = psp.tile([P, NSEQ], f32, tag="ps", name=f"xpt{b}_{s}_{do}")
                    for no in range(SCH):
                        nc.tensor.transpose(
                            pt[:, no * P : (no + 1) * P],
                            xnd[:, no, do * P : (do + 1) * P],
                            ident,
                        )
                    nc.vector.tensor_copy(xT[:, do, :], pt)
                    nc.scalar.copy(xTb[:, do, :], pt)
                xs[s] = (xT, xTb)

            er_scale = ec_scale = None
            Er = Ec = None
            for l in range(L):
                if scales_r[l] != er_scale:
                    Er = get_Er(scales_r[l])
                    er_scale = scales_r[l]
                if scales_c[l] != ec_scale:
                    Ec = get_Ec(scales_c[l])
                    ec_scale = scales_c[l]
                last = l == L - 1
                nr, ncl = enc_pair(l, xs, Er, Ec, last)
                xs[0], xs[1] = nr, ncl

            for s in (0, 1):
                nx = xs[s][0]
                ond = opool.tile([P, SCH, D], f32, tag="ond", name="ond")
                for no in range(SCH):
                    ops_ = psp.tile([P, D], f32, tag="ps", name=f"ops{b}_{s}_{no}")
                    for do in range(DCH):
                        nc.tensor.transpose(
                            ops_[:, do * P : (do + 1) * P],
                            nx[:, do, no * P : (no + 1) * P],
                            ident,
                        )
                    nc.vector.tensor_copy(ond[:, no, :], ops_)
                nc.sync.dma_start(
                    out_d[s, b].rearrange("(no ni) d -> ni no d", ni=P), ond
                )

    nc.compile()
    return nc


def _get_compiled(scales_r, scales_c, flags):
    from concourse.bass_interp import get_hw_module

    key = (scales_r, scales_c, flags)
    if key not in _CACHE:
        nc = _build(scales_r, scales_c, **dict(flags))
        nc.m = get_hw_module(nc.m)
        _CACHE[key] = nc
    return _CACHE[key]


def kernel(**inputs) -> np.ndarray:
    global LAST_RESULT
    from concourse import bass_utils

    def f32c(x):
        return np.ascontiguousarray(np.asarray(x, dtype=np.float32))

    log_scale = float(np.asarray(inputs["log_scale"]))
    alpha = np.asarray(inputs["alpha"], dtype=np.float64)
    scales_r = tuple(float(-log_scale * alpha[l, 0]) for l in range(L))
    scales_c = tuple(float(-log_scale * alpha[l, 1]) for l in range(L))

    flags = (
        ("unit_g1", bool(np.all(np.asarray(inputs["g1"]) == 1.0))),
        ("zero_be1", bool(np.all(np.asarray(inputs["be1"]) == 0.0))),
        ("unit_g2", bool(np.all(np.asarray(inputs["g2"]) == 1.0))),
        ("zero_be2", bool(np.all(np.asarray(inputs["be2"]) == 0.0))),
        ("zero_b1", bool(np.all(np.asarray(inputs["b1"]) == 0.0))),
    )
    nc = _get_compiled(scales_r, scales_c, flags)

    shard_names = ("row_emb", "col_emb", "cost_mat")
    rep_names = ("Wq", "Wk", "Wv", "g1", "be1", "W1", "b1", "W2", "g2", "be2")
    rep = {k: f32c(inputs[k]) for k in rep_names}
    in_maps = []
    for c in range(NCORES):
        m = dict(rep)
        for k in shard_names:
            m[k] = f32c(np.asarray(inputs[k])[c * BLOC : (c + 1) * BLOC])
        in_maps.append(m)

    res = bass_utils.run_bass_kernel_spmd(nc, in_maps, core_ids=list(range(NCORES)))
    LAST_RESULT = res
    out = np.concatenate([res.results[c]["out"] for c in range(NCORES)], axis=1)
    return out


# revision 46
# speedup vs baseline: 1.0007x; 1.0007x over previous
"""ATSP encoder (5-layer dual-stream AFT transformer) on 8 TRN2 NeuronCores.

Sharding: data-parallel over batch B=128 -> 16 items per core, params
replicated. Per core the whole network runs out of SBUF per batch item.

Layout: residual streams are kept transposed [D(part), seq(free)] so that
instance-norm (reduce over seq) is a free-axis bn_stats, the per-channel
affine is per-partition, and FF/projection matmuls contract naturally.
k/v are produced in [seq, D] (activation as matmul lhsT), and the AFT GEMMs
compute numT/denT = lhsT(ekv|ek).T @ rhs(E^T) straight back into the
transposed layout -- no transposes anywhere in the layer loop.

Matmuls run in bf16 (fp32 PSUM accumulation); the residual stream stays
fp32. alpha/log_scale are folded into compile-time exp() scales and b2 is
dropped (a per-channel constant shift cancels exactly in instance norm).
"""

import numpy as np

B, NSEQ, D, F, L = 128, 512, 256, 512, 5
NCORES = 8
BLOC = B // NCORES
P = 128
DCH, SCH, FCH = D // P, NSEQ // P, F // P
EPS = 1e-5

_CACHE: dict = {}
LAST_RESULT = None


def _build(scales_r, scales_c, bloc=BLOC, enable_asserts=False, num_devices=NCORES,
           unit_g1=False, zero_be1=False, unit_g2=False, zero_be2=False, zero_b1=False):
    from contextlib import ExitStack

    import concourse.bacc as bacc
    import concourse.mybir as mybir
    import concourse.tile as tile
    from concourse.masks import make_identity

    dt = mybir.dt
    AF = mybir.ActivationFunctionType
    OP = mybir.AluOpType
    f32 = dt.float32
    bf16 = dt.bfloat16

    nc = bacc.Bacc(
        "TRN2",
        target_bir_lowering=False,
        debug=False,
        enable_asserts=enable_asserts,
        num_devices=num_devices,
    )

    row_d = nc.dram_tensor("row_emb", [bloc, NSEQ, D], f32, kind="ExternalInput").ap()
    col_d = nc.dram_tensor("col_emb", [bloc, NSEQ, D], f32, kind="ExternalInput").ap()
    cost_d = nc.dram_tensor("cost_mat", [bloc, NSEQ, NSEQ], f32, kind="ExternalInput").ap()
    wq_d = nc.dram_tensor("Wq", [L, 2, D, D], f32, kind="ExternalInput").ap()
    wk_d = nc.dram_tensor("Wk", [L, 2, D, D], f32, kind="ExternalInput").ap()
    wv_d = nc.dram_tensor("Wv", [L, 2, D, D], f32, kind="ExternalInput").ap()
    g1_d = nc.dram_tensor("g1", [L, 2, D], f32, kind="ExternalInput").ap()
    be1_d = nc.dram_tensor("be1", [L, 2, D], f32, kind="ExternalInput").ap()
    w1_d = nc.dram_tensor("W1", [L, 2, D, F], f32, kind="ExternalInput").ap()
    b1_d = nc.dram_tensor("b1", [L, 2, F], f32, kind="ExternalInput").ap()
    w2_d = nc.dram_tensor("W2", [L, 2, F, D], f32, kind="ExternalInput").ap()
    g2_d = nc.dram_tensor("g2", [L, 2, D], f32, kind="ExternalInput").ap()
    be2_d = nc.dram_tensor("be2", [L, 2, D], f32, kind="ExternalInput").ap()
    out_d = nc.dram_tensor("out", [2, bloc, NSEQ, D], f32, kind="ExternalOutput").ap()

    with tile.TileContext(nc) as tc, ExitStack() as ctx:
        # Pre-load the combined exp+ln activation table set once. Every
        # activation in this kernel (Exp, Ln, Relu, Copy, Identity) lives in
        # this one set, so the fixpoint table-load pass inserts no further
        # ACT_TABLE_LOADs (saves ~4 reloads x ~1.3us per layer-side).
        from concourse.hw_specs import get_activation_tables

        table_names = list(get_activation_tables(nc.m.arch))
        combined_id = table_names.index("natural_log_exp_and_others")
        nc.scalar.add_instruction(
            mybir.InstLoadActFuncSet(
                act_func_set_id=combined_id,
                name=nc.get_next_instruction_name(),
                ins=[],
                outs=[],
            )
        )

        consts = ctx.enter_context(tc.tile_pool(name="consts", bufs=1))
        wpool = ctx.enter_context(tc.tile_pool(name="wpool", bufs=1))

        ident = consts.tile([P, P], f32)
        make_identity(nc, ident)
        epsc = consts.tile([P, 1], f32)
        nc.vector.memset(epsc, EPS)

        g1a = consts.tile([P, L * 2 * DCH], f32)
        nc.sync.dma_start(g1a, g1_d.rearrange("l s (c ci) -> ci (l s c)", ci=P))
        be1a = consts.tile([P, L * 2 * DCH], f32)
        nc.sync.dma_start(be1a, be1_d.rearrange("l s (c ci) -> ci (l s c)", ci=P))
        g2a = consts.tile([P, L * 2 * DCH], f32)
        nc.sync.dma_start(g2a, g2_d.rearrange("l s (c ci) -> ci (l s c)", ci=P))
        be2a = consts.tile([P, L * 2 * DCH], f32)
        nc.sync.dma_start(be2a, be2_d.rearrange("l s (c ci) -> ci (l s c)", ci=P))
        b1a = consts.tile([P, L * 2 * FCH], f32)
        nc.sync.dma_start(b1a, b1_d.rearrange("l s (c ci) -> ci (l s c)", ci=P))

        with tc.tile_pool(name="wstage", bufs=2) as wstage:

            def load_w(dram_ap, ko_cnt, o_dim, name):
                stgt = wstage.tile([P, L * 2 * ko_cnt, o_dim], f32, tag="wstg", name=f"stg_{name}")
                nc.sync.dma_start(
                    stgt, dram_ap.rearrange("l s (ko ki) o -> ki (l s ko) o", ki=P)
                )
                wb = wpool.tile([P, L * 2 * ko_cnt, o_dim], bf16, name=name)
                nc.vector.tensor_copy(wb, stgt)
                return wb

            WqB = load_w(wq_d, DCH, D, "WqB")
            W1B = load_w(w1_d, DCH, F, "W1B")
            W2B = load_w(w2_d, FCH, D, "W2B")
            # pack [Wk | Wv] along the output dim: one N=512 rhs for the
            # k|v matmuls (half the matmul/LDWEIGHTS instruction count)
            WkvB = wpool.tile([P, L * 2 * DCH, 2 * D], bf16, name="WkvB")
            for w_d, off in ((wk_d, 0), (wv_d, D)):
                stgt = wstage.tile(
                    [P, L * 2 * DCH, D], f32, tag="wstg", name=f"stg_kv{off}"
                )
                nc.sync.dma_start(
                    stgt, w_d.rearrange("l s (ko ki) o -> ki (l s ko) o", ki=P)
                )
                nc.vector.tensor_copy(WkvB[:, :, off : off + D], stgt)

        cmp_ = ctx.enter_context(tc.tile_pool(name="cmpool", bufs=1))
        epool = ctx.enter_context(tc.tile_pool(name="epool", bufs=2))
        stg = ctx.enter_context(tc.tile_pool(name="stgpool", bufs=2))
        opool = ctx.enter_context(tc.tile_pool(name="opool", bufs=1))
        ipool = ctx.enter_context(tc.tile_pool(name="ipool", bufs=1))
        kvp = ctx.enter_context(tc.tile_pool(name="kvp", bufs=4))
        xp = ctx.enter_context(tc.tile_pool(name="xp", bufs=3))
        strm = ctx.enter_context(tc.tile_pool(name="strm", bufs=2))
        tpool = ctx.enter_context(tc.tile_pool(name="tpool", bufs=2))
        ttp = ctx.enter_context(tc.tile_pool(name="ttp", bufs=4))
        spool = ctx.enter_context(tc.tile_pool(name="spool", bufs=4))
        psp = ctx.enter_context(tc.tile_pool(name="psp", bufs=8, space="PSUM"))

        INV_N = 1.0 / NSEQ

        def in_norm(xin, xsum, ga, bea, unit_g, zero_be, lsi, outf, outb):
            """xin [P, DCH, NSEQ], xsum [P, DCH] = per-partition sums of xin
            (accumulated by the producing adds). Variance via E[x^2]-mu^2:
            the sum of squares comes from an ACT Square pass with accum_out,
            keeping the whole stats chain off the DVE critical path."""
            qsum = spool.tile([P, DCH], f32, tag="qsum", name="qsum")
            for do in range(DCH):
                ssq = ttp.tile([P, NSEQ], f32, tag="tt", name="ssq")
                if do == 0:
                    nc.scalar.activation(
                        ssq, xin[:, do, :], AF.Square,
                        accum_out=qsum[:, do : do + 1],
                    )
                else:
                    # second chunk's sum-of-squares on DVE, in parallel with
                    # the ACT square of the first chunk
                    nc.vector.scalar_tensor_tensor(
                        ssq, xin[:, do, :], 0.0, xin[:, do, :], OP.add, OP.mult,
                        accum_out=qsum[:, do : do + 1],
                    )
            # b = sumsq - xsum^2/N  (=> var = b/N)
            a = spool.tile([P, DCH], f32, tag="a", name="a")
            bvar = spool.tile([P, DCH], f32, tag="bvar", name="bvar")
            with tc.high_priority(offset=24):
                nc.vector.scalar_tensor_tensor(a, xsum, INV_N, xsum, OP.mult, OP.mult)
                nc.vector.tensor_sub(bvar, qsum, a)
            # 1/sqrt(var+eps) = exp(-0.5*ln(b/N+eps)): stays in the exp/ln
            # ACT table set, so the whole kernel runs on one resident table.
            lnv = spool.tile([P, DCH], f32, tag="lnv", name="lnv")
            nc.scalar.activation(lnv, bvar, AF.Ln, bias=epsc, scale=INV_N)
            rs = spool.tile([P, DCH], f32, tag="rs", name="rs")
            nc.scalar.activation(rs, lnv, AF.Exp, scale=-0.5)
            if unit_g:
                s1 = rs
            else:
                s1 = spool.tile([P, DCH], f32, tag="s1", name="s1")
                nc.vector.tensor_mul(s1, rs, ga[:, lsi * DCH : (lsi + 1) * DCH])
            bb = spool.tile([P, DCH], f32, tag="bb", name="bb")
            if zero_be:
                # bb = -mean * s1 = -(xsum/N) * s1
                with tc.high_priority(offset=24):
                    nc.vector.scalar_tensor_tensor(bb, xsum, -INV_N, s1, OP.mult, OP.mult)
            else:
                ms = spool.tile([P, DCH], f32, tag="ms", name="ms")
                nc.vector.scalar_tensor_tensor(ms, xsum, -INV_N, s1, OP.mult, OP.mult)
                nc.vector.tensor_add(bb, bea[:, lsi * DCH : (lsi + 1) * DCH], ms)
            # split the applies across DVE and GPSIMD so the two chunks run
            # in parallel instead of serializing on one engine
            napply = 0
            for do in range(DCH):
                for out in (outf, outb):
                    if out is None:
                        continue
                    eng = nc.vector if napply % 2 == 0 else nc.gpsimd
                    eng.tensor_scalar(
                        out[:, do, :], xin[:, do, :],
                        s1[:, do : do + 1], bb[:, do : do + 1],
                        OP.mult, OP.add,
                    )
                    napply += 1

        def q_stage(lsi, xTb):
            # q -> eq = exp(-q) in [D, n]; sigmoid(q)*num/den is computed as
            # num / (den * (1+eq)), so no sigmoid table set is ever needed.
            u = kvp.tile([P, DCH, NSEQ], bf16, tag="u", name="u")
            for mo in range(DCH):
                qps = psp.tile([P, NSEQ], f32, tag="ps", name=f"qps{lsi}_{mo}")
                for ko in range(DCH):
                    nc.tensor.matmul(
                        qps,
                        WqB[:, lsi * DCH + ko, mo * P : (mo + 1) * P],
                        xTb[:, ko, :],
                        start=(ko == 0), stop=(ko == DCH - 1),
                    )
                nc.scalar.activation(u[:, mo, :], qps, AF.Exp, scale=-1.0)
            return u

        def kv_stage(lsi, yTb):
            # k|v packed per seq-chunk, in [m, D]
            ek = kvp.tile([P, SCH, D], bf16, tag="ek", name="ek")
            ekv = kvp.tile([P, SCH, D], bf16, tag="ekv", name="ekv")
            for sc in range(SCH):
                kvps = psp.tile([P, NSEQ], f32, tag="ps", name=f"kvps{lsi}_{sc}")
                for ko in range(DCH):
                    nc.tensor.matmul(
                        kvps,
                        yTb[:, ko, sc * P : (sc + 1) * P],
                        WkvB[:, lsi * DCH + ko, :],
                        start=(ko == 0), stop=(ko == DCH - 1),
                    )
                with tc.high_priority(offset=16):
                    nc.scalar.activation(ek[:, sc, :], kvps[:, 0:D], AF.Exp)
                    nc.vector.tensor_mul(ekv[:, sc, :], kvps[:, D : 2 * D], ek[:, sc, :])
            return ek, ekv

        def aft_stage(lsi, u, ek, ekv, E, xT):
            # AFT: numT/denT [D, n] = (ekv|ek).T @ E^T, then combine + residual
            x1 = xp.tile([P, DCH, NSEQ], f32, tag="x1", name="x1")
            x1sum = spool.tile([P, DCH], f32, tag="xsum", name="x1sum")
            for do in range(DCH):
                # den first: the (1+eq)*den + reciprocal chain runs on DVE
                # while the num matmuls stream on PE
                dps = psp.tile([P, NSEQ], f32, tag="ps", name=f"dps{lsi}_{do}")
                for sc in range(SCH):
                    nc.tensor.matmul(
                        dps, ek[:, sc, do * P : (do + 1) * P], E[:, sc, :],
                        start=(sc == 0), stop=(sc == SCH - 1),
                    )
                dd = ttp.tile([P, NSEQ], f32, tag="tt", name="dd")
                # dd = (eq + 1) * den  -- folds the sigmoid denominator in
                with tc.high_priority(offset=16):
                    nc.vector.scalar_tensor_tensor(dd, u[:, do, :], 1.0, dps, OP.add, OP.mult)
                rdd = ttp.tile([P, NSEQ], f32, tag="tt", name="rdd")
                nc.vector.reciprocal_approx_fast(rdd, dd)
                nps = psp.tile([P, NSEQ], f32, tag="ps", name=f"nps{lsi}_{do}")
                for sc in range(SCH):
                    nc.tensor.matmul(
                        nps, ekv[:, sc, do * P : (do + 1) * P], E[:, sc, :],
                        start=(sc == 0), stop=(sc == SCH - 1),
                    )
                t = ttp.tile([P, NSEQ], f32, tag="tt", name="t")
                with tc.high_priority(offset=16):
                    nc.vector.tensor_mul(t, nps, rdd)
                nc.vector.scalar_tensor_tensor(
                    x1[:, do, :], t, 0.0, xT[:, do, :], OP.add, OP.add,
                    accum_out=x1sum[:, do : do + 1],
                )
            return x1, x1sum

        def in1_stage(lsi, x1, x1sum):
            # h1 is kept only in bf16: it feeds the ff1 matmuls directly and
            # the ff2 residual add (bf16 rounding there is within budget)
            h1b = tpool.tile([P, DCH, NSEQ], bf16, tag="h1b", name="h1b")
            in_norm(x1, x1sum, g1a, be1a, unit_g1, zero_be1, lsi, None, h1b)
            return h1b

        def ff1_stage(lsi, h1b):
            ff1b = tpool.tile([P, FCH, NSEQ], bf16, tag="ff1b", name="ff1b")
            for fo in range(FCH):
                fps = psp.tile([P, NSEQ], f32, tag="ps", name=f"fps{lsi}_{fo}")
                for ko in range(DCH):
                    nc.tensor.matmul(
                        fps,
                        W1B[:, lsi * DCH + ko, fo * P : (fo + 1) * P],
                        h1b[:, ko, :],
                        start=(ko == 0), stop=(ko == DCH - 1),
                    )
                if zero_b1:
                    nc.scalar.activation(ff1b[:, fo, :], fps, AF.Relu, bias=0.0)
                else:
                    nc.scalar.activation(
                        ff1b[:, fo, :], fps, AF.Relu,
                        bias=b1a[:, lsi * FCH + fo : lsi * FCH + fo + 1],
                    )
            return ff1b

        def ff2_stage(lsi, ff1b, h1b):
            x2 = xp.tile([P, DCH, NSEQ], f32, tag="x1", name="x2")
            x2sum = spool.tile([P, DCH], f32, tag="xsum", name="x2sum")
            for do in range(DCH):
                f2ps = psp.tile([P, NSEQ], f32, tag="ps", name=f"f2ps{lsi}_{do}")
                for ko in range(FCH):
                    nc.tensor.matmul(
                        f2ps,
                        W2B[:, lsi * FCH + ko, do * P : (do + 1) * P],
                        ff1b[:, ko, :],
                        start=(ko == 0), stop=(ko == FCH - 1),
                    )
                nc.vector.scalar_tensor_tensor(
                    x2[:, do, :], f2ps, 0.0, h1b[:, do, :], OP.add, OP.add,
                    accum_out=x2sum[:, do : do + 1],
                )
            return x2, x2sum

        def in2_stage(lsi, x2, x2sum, last):
            s = lsi % 2
            nx = strm.tile([P, DCH, NSEQ], f32, tag=f"x{s}", name=f"x{s}")
            nxb = None
            if not last:
                nxb = strm.tile([P, DCH, NSEQ], bf16, tag=f"xb{s}", name=f"xb{s}")
            in_norm(x2, x2sum, g2a, be2a, unit_g2, zero_be2, lsi, nx, nxb)
            return nx, nxb

        def enc_pair(l, xs, Er, Ec, last):
            # Interleave the two independent sides of a layer, with the col
            # side staggered ~1.5 stages behind the row side so the two
            # serial norm chains never coincide -- one side's matmuls keep
            # PE fed while the other side's stats/apply chain runs.
            lsr, lsc = l * 2, l * 2 + 1
            (xr, xrb), (xc, xcb) = xs[0], xs[1]
            ur = q_stage(lsr, xrb)
            ekr, ekvr = kv_stage(lsr, xcb)
            uc = q_stage(lsc, xcb)
            x1r, x1sr = aft_stage(lsr, ur, ekr, ekvr, Er, xr)
            ekc, ekvc = kv_stage(lsc, xrb)
            h1br = in1_stage(lsr, x1r, x1sr)
            x1c, x1sc = aft_stage(lsc, uc, ekc, ekvc, Ec, xc)
            f1r = ff1_stage(lsr, h1br)
            h1bc = in1_stage(lsc, x1c, x1sc)
            x2r, x2sr = ff2_stage(lsr, f1r, h1br)
            f1c = ff1_stage(lsc, h1bc)
            nr = in2_stage(lsr, x2r, x2sr, last)
            x2c, x2sc = ff2_stage(lsc, f1c, h1bc)
            ncl = in2_stage(lsc, x2c, x2sc, last)
            return nr, ncl

        for b in range(bloc):
            cm = cmp_.tile([P, SCH, NSEQ], f32, tag="cm", name="cm")
            nc.sync.dma_start(cm, cost_d[b].rearrange("(no ni) m -> ni no m", ni=P))

            def get_Ec(scale, b=b):
                Ec = epool.tile([P, SCH, NSEQ], bf16, tag="Ec", name="Ec")
                for no in range(SCH):
                    nc.scalar.activation(Ec[:, no, :], cm[:, no, :], AF.Exp, scale=scale)
                return Ec

            def get_Er(scale, b=b):
                Er = epool.tile([P, SCH, NSEQ], bf16, tag="Er", name="Er")
                for mo in range(SCH):
                    pt = psp.tile([P, NSEQ], f32, tag="ps", name=f"ept{b}_{mo}")
                    for no in range(SCH):
                        nc.tensor.transpose(
                            pt[:, no * P : (no + 1) * P],
                            cm[:, no, mo * P : (mo + 1) * P],
                            ident,
                        )
                    nc.scalar.activation(Er[:, mo, :], pt, AF.Exp, scale=scale)
                return Er

            xs = {}
            for s, src in ((0, row_d), (1, col_d)):
                xnd = stg.tile([P, SCH, D], f32, tag="xnd", name="xnd")
                nc.sync.dma_start(xnd, src[b].rearrange("(no ni) d -> ni no d", ni=P))
                # dedicated tags for the item-initial stream tiles so the
                # next item's input staging never rotates against this
                # item's per-layer stream tiles (cross-item overlap)
                xT = ipool.tile([P, DCH, NSEQ], f32, tag=f"xi{s}", name=f"xi{s}")
                xTb = ipool.tile([P, DCH, NSEQ], bf16, tag=f"xbi{s}", name=f"xbi{s}")
                for do in range(DCH):
                    pt = psp.tile([P, NSEQ], f32, tag="ps", name=f"xpt{b}_{s}_{do}")
                    for no in range(SCH):
                        nc.tensor.transpose(
                            pt[:, no * P : (no + 1) * P],
                            xnd[:, no, do * P : (do + 1) * P],
                            ident,
                        )
                    nc.vector.tensor_copy(xT[:, do, :], pt)
                    nc.scalar.copy(xTb[:, do, :], pt)
                xs[s] = (xT, xTb)

            er_scale = ec_scale = None
            Er = Ec = None
            for l in range(L):
                if scales_r[l] != er_scale:
                    Er = get_Er(scales_r[l])
                    er_scale = scales_r[l]
                if scales_c[l] != ec_scale:
                    Ec = get_Ec(scales_c[l])
                    ec_scale = scales_c[l]
                last = l == L - 1
                nr, ncl = enc_pair(l, xs, Er, Ec, last)
                xs[0], xs[1] = nr, ncl

            for s in (0, 1):
                nx = xs[s][0]
                ond = opool.tile([P, SCH, D], f32, tag="ond", name="ond")
                for no in range(SCH):
                    ops_ = psp.tile([P, D], f32, tag="ps", name=f"ops{b}_{s}_{no}")
                    for do in range(DCH):
                        nc.tensor.transpose(
                            ops_[:, do * P : (do + 1) * P],
                            nx[:, do, no * P : (no + 1) * P],
                            ident,
                        )
                    nc.vector.tensor_copy(ond[:, no, :], ops_)
                nc.sync.dma_start(
                    out_d[s, b].rearrange("(no ni) d -> ni no d", ni=P), ond
                )

    nc.compile()
    return nc


def _get_compiled(scales_r, scales_c, flags):
    from concourse.bass_interp import get_hw_module

    key = (scales_r, scales_c, flags)
    if key not in _CACHE:
        nc = _build(scales_r, scales_c, **dict(flags))
        nc.m = get_hw_module(nc.m)
        _CACHE[key] = nc
    return _CACHE[key]


def kernel(**inputs) -> np.ndarray:
    global LAST_RESULT
    from concourse import bass_utils

    def f32c(x):
        return np.ascontiguousarray(np.asarray(x, dtype=np.float32))

    log_scale = float(np.asarray(inputs["log_scale"]))
    alpha = np.asarray(inputs["alpha"], dtype=np.float64)
    scales_r = tuple(float(-log_scale * alpha[l, 0]) for l in range(L))
    scales_c = tuple(float(-log_scale * alpha[l, 1]) for l in range(L))

    flags = (
        ("unit_g1", bool(np.all(np.asarray(inputs["g1"]) == 1.0))),
        ("zero_be1", bool(np.all(np.asarray(inputs["be1"]) == 0.0))),
        ("unit_g2", bool(np.all(np.asarray(inputs["g2"]) == 1.0))),
        ("zero_be2", bool(np.all(np.asarray(inputs["be2"]) == 0.0))),
        ("zero_b1", bool(np.all(np.asarray(inputs["b1"]) == 0.0))),
    )
    nc = _get_compiled(scales_r, scales_c, flags)

    shard_names = ("row_emb", "col_emb", "cost_mat")
    rep_names = ("Wq", "Wk", "Wv", "g1", "be1", "W1", "b1", "W2", "g2", "be2")
    rep = {k: f32c(inputs[k]) for k in rep_names}
    in_maps = []
    for c in range(NCORES):
        m = dict(rep)
        for k in shard_names:
            m[k] = f32c(np.asarray(inputs[k])[c * BLOC : (c + 1) * BLOC])
        in_maps.append(m)

    res = bass_utils.run_bass_kernel_spmd(nc, in_maps, core_ids=list(range(NCORES)))
    LAST_RESULT = res
    out = np.concatenate([res.results[c]["out"] for c in range(NCORES)], axis=1)
    return out


# revision 47
# speedup vs baseline: 1.2019x; 1.2011x over previous
"""ATSP encoder (5-layer dual-stream AFT transformer) on 8 TRN2 NeuronCores.

Sharding: data-parallel over batch B=128 -> 16 items per core, params
replicated. Per core the whole network runs out of SBUF per batch item.

Layout: residual streams are kept transposed [D(part), seq(free)] so that
instance-norm (reduce over seq) is a free-axis bn_stats, the per-channel
affine is per-partition, and FF/projection matmuls contract naturally.
k/v are produced in [seq, D] (activation as matmul lhsT), and the AFT GEMMs
compute numT/denT = lhsT(ekv|ek).T @ rhs(E^T) straight back into the
transposed layout -- no transposes anywhere in the layer loop.

Matmuls run in bf16 (fp32 PSUM accumulation); the residual stream stays
fp32. alpha/log_scale are folded into compile-time exp() scales and b2 is
dropped (a per-channel constant shift cancels exactly in instance norm).
"""

import numpy as np

B, NSEQ, D, F, L = 128, 512, 256, 512, 5
NCORES = 8
BLOC = B // NCORES
P = 128
DCH, SCH, FCH = D // P, NSEQ // P, F // P
EPS = 1e-5

_CACHE: dict = {}
LAST_RESULT = None


def _build(scales_r, scales_c, bloc=BLOC, enable_asserts=False, num_devices=NCORES,
           unit_g1=False, zero_be1=False, unit_g2=False, zero_be2=False, zero_b1=False):
    from contextlib import ExitStack

    import concourse.bacc as bacc
    import concourse.mybir as mybir
    import concourse.tile as tile
    from concourse.masks import make_identity

    dt = mybir.dt
    AF = mybir.ActivationFunctionType
    OP = mybir.AluOpType
    f32 = dt.float32
    bf16 = dt.bfloat16

    nc = bacc.Bacc(
        "TRN2",
        target_bir_lowering=False,
        debug=False,
        enable_asserts=enable_asserts,
        num_devices=num_devices,
    )

    row_d = nc.dram_tensor("row_emb", [bloc, NSEQ, D], f32, kind="ExternalInput").ap()
    col_d = nc.dram_tensor("col_emb", [bloc, NSEQ, D], f32, kind="ExternalInput").ap()
    cost_d = nc.dram_tensor("cost_mat", [bloc, NSEQ, NSEQ], f32, kind="ExternalInput").ap()
    wq_d = nc.dram_tensor("Wq", [L, 2, D, D], f32, kind="ExternalInput").ap()
    wk_d = nc.dram_tensor("Wk", [L, 2, D, D], f32, kind="ExternalInput").ap()
    wv_d = nc.dram_tensor("Wv", [L, 2, D, D], f32, kind="ExternalInput").ap()
    g1_d = nc.dram_tensor("g1", [L, 2, D], f32, kind="ExternalInput").ap()
    be1_d = nc.dram_tensor("be1", [L, 2, D], f32, kind="ExternalInput").ap()
    w1_d = nc.dram_tensor("W1", [L, 2, D, F], f32, kind="ExternalInput").ap()
    b1_d = nc.dram_tensor("b1", [L, 2, F], f32, kind="ExternalInput").ap()
    w2_d = nc.dram_tensor("W2", [L, 2, F, D], f32, kind="ExternalInput").ap()
    g2_d = nc.dram_tensor("g2", [L, 2, D], f32, kind="ExternalInput").ap()
    be2_d = nc.dram_tensor("be2", [L, 2, D], f32, kind="ExternalInput").ap()
    out_d = nc.dram_tensor("out", [2, bloc, NSEQ, D], f32, kind="ExternalOutput").ap()

    with tile.TileContext(nc) as tc, ExitStack() as ctx:
        # Pre-load the combined exp+ln activation table set once. Every
        # activation in this kernel (Exp, Ln, Relu, Copy, Identity) lives in
        # this one set, so the fixpoint table-load pass inserts no further
        # ACT_TABLE_LOADs (saves ~4 reloads x ~1.3us per layer-side).
        from concourse.hw_specs import get_activation_tables

        table_names = list(get_activation_tables(nc.m.arch))
        combined_id = table_names.index("natural_log_exp_and_others")
        nc.scalar.add_instruction(
            mybir.InstLoadActFuncSet(
                act_func_set_id=combined_id,
                name=nc.get_next_instruction_name(),
                ins=[],
                outs=[],
            )
        )

        consts = ctx.enter_context(tc.tile_pool(name="consts", bufs=1))
        wpool = ctx.enter_context(tc.tile_pool(name="wpool", bufs=1))

        ident = consts.tile([P, P], f32)
        make_identity(nc, ident)
        epsc = consts.tile([P, 1], f32)
        nc.vector.memset(epsc, EPS)

        g1a = consts.tile([P, L * 2 * DCH], f32)
        nc.sync.dma_start(g1a, g1_d.rearrange("l s (c ci) -> ci (l s c)", ci=P))
        be1a = consts.tile([P, L * 2 * DCH], f32)
        nc.sync.dma_start(be1a, be1_d.rearrange("l s (c ci) -> ci (l s c)", ci=P))
        g2a = consts.tile([P, L * 2 * DCH], f32)
        nc.sync.dma_start(g2a, g2_d.rearrange("l s (c ci) -> ci (l s c)", ci=P))
        be2a = consts.tile([P, L * 2 * DCH], f32)
        nc.sync.dma_start(be2a, be2_d.rearrange("l s (c ci) -> ci (l s c)", ci=P))
        b1a = consts.tile([P, L * 2 * FCH], f32)
        nc.sync.dma_start(b1a, b1_d.rearrange("l s (c ci) -> ci (l s c)", ci=P))

        with tc.tile_pool(name="wstage", bufs=2) as wstage:

            def load_w(dram_ap, ko_cnt, o_dim, name):
                stgt = wstage.tile([P, L * 2 * ko_cnt, o_dim], f32, tag="wstg", name=f"stg_{name}")
                nc.sync.dma_start(
                    stgt, dram_ap.rearrange("l s (ko ki) o -> ki (l s ko) o", ki=P)
                )
                wb = wpool.tile([P, L * 2 * ko_cnt, o_dim], bf16, name=name)
                nc.vector.tensor_copy(wb, stgt)
                return wb

            WqB = load_w(wq_d, DCH, D, "WqB")
            W1B = load_w(w1_d, DCH, F, "W1B")
            W2B = load_w(w2_d, FCH, D, "W2B")
            # pack [Wk | Wv] along the output dim: one N=512 rhs for the
            # k|v matmuls (half the matmul/LDWEIGHTS instruction count)
            WkvB = wpool.tile([P, L * 2 * DCH, 2 * D], bf16, name="WkvB")
            for w_d, off in ((wk_d, 0), (wv_d, D)):
                stgt = wstage.tile(
                    [P, L * 2 * DCH, D], f32, tag="wstg", name=f"stg_kv{off}"
                )
                nc.sync.dma_start(
                    stgt, w_d.rearrange("l s (ko ki) o -> ki (l s ko) o", ki=P)
                )
                nc.vector.tensor_copy(WkvB[:, :, off : off + D], stgt)

        cmp_ = ctx.enter_context(tc.tile_pool(name="cmpool", bufs=1))
        epool = ctx.enter_context(tc.tile_pool(name="epool", bufs=2))
        stg = ctx.enter_context(tc.tile_pool(name="stgpool", bufs=2))
        strm = ctx.enter_context(tc.tile_pool(name="strm", bufs=2))
        tpool = ctx.enter_context(tc.tile_pool(name="tpool", bufs=2))
        ttp = ctx.enter_context(tc.tile_pool(name="ttp", bufs=4))
        spool = ctx.enter_context(tc.tile_pool(name="spool", bufs=4))
        psp = ctx.enter_context(tc.tile_pool(name="psp", bufs=8, space="PSUM"))

        INV_N = 1.0 / NSEQ

        def in_norm(xin, xsum, ga, bea, unit_g, zero_be, lsi, outf, outb):
            """xin [P, DCH, NSEQ], xsum [P, DCH] = per-partition sums of xin
            (accumulated by the producing adds). Variance via E[x^2]-mu^2:
            the sum of squares comes from an ACT Square pass with accum_out,
            keeping the whole stats chain off the DVE critical path."""
            qsum = spool.tile([P, DCH], f32, tag="qsum", name="qsum")
            for do in range(DCH):
                ssq = ttp.tile([P, NSEQ], f32, tag="tt", name="ssq")
                if do == 0:
                    nc.scalar.activation(
                        ssq, xin[:, do, :], AF.Square,
                        accum_out=qsum[:, do : do + 1],
                    )
                else:
                    # second chunk's sum-of-squares on DVE, in parallel with
                    # the ACT square of the first chunk
                    nc.vector.scalar_tensor_tensor(
                        ssq, xin[:, do, :], 0.0, xin[:, do, :], OP.add, OP.mult,
                        accum_out=qsum[:, do : do + 1],
                    )
            # b = sumsq - xsum^2/N  (=> var = b/N)
            a = spool.tile([P, DCH], f32, tag="a", name="a")
            bvar = spool.tile([P, DCH], f32, tag="bvar", name="bvar")
            with tc.high_priority(offset=24):
                nc.vector.scalar_tensor_tensor(a, xsum, INV_N, xsum, OP.mult, OP.mult)
                nc.vector.tensor_sub(bvar, qsum, a)
            # 1/sqrt(var+eps) = exp(-0.5*ln(b/N+eps)): stays in the exp/ln
            # ACT table set, so the whole kernel runs on one resident table.
            lnv = spool.tile([P, DCH], f32, tag="lnv", name="lnv")
            nc.scalar.activation(lnv, bvar, AF.Ln, bias=epsc, scale=INV_N)
            rs = spool.tile([P, DCH], f32, tag="rs", name="rs")
            nc.scalar.activation(rs, lnv, AF.Exp, scale=-0.5)
            if unit_g:
                s1 = rs
            else:
                s1 = spool.tile([P, DCH], f32, tag="s1", name="s1")
                nc.vector.tensor_mul(s1, rs, ga[:, lsi * DCH : (lsi + 1) * DCH])
            bb = spool.tile([P, DCH], f32, tag="bb", name="bb")
            if zero_be:
                # bb = -mean * s1 = -(xsum/N) * s1
                with tc.high_priority(offset=24):
                    nc.vector.scalar_tensor_tensor(bb, xsum, -INV_N, s1, OP.mult, OP.mult)
            else:
                ms = spool.tile([P, DCH], f32, tag="ms", name="ms")
                nc.vector.scalar_tensor_tensor(ms, xsum, -INV_N, s1, OP.mult, OP.mult)
                nc.vector.tensor_add(bb, bea[:, lsi * DCH : (lsi + 1) * DCH], ms)
            # split the applies across DVE and GPSIMD so the two chunks run
            # in parallel instead of serializing on one engine
            napply = 0
            for do in range(DCH):
                for out in (outf, outb):
                    if out is None:
                        continue
                    eng = nc.vector if napply % 2 == 0 else nc.gpsimd
                    eng.tensor_scalar(
                        out[:, do, :], xin[:, do, :],
                        s1[:, do : do + 1], bb[:, do : do + 1],
                        OP.mult, OP.add,
                    )
                    napply += 1

        def q_stage(lsi, xTb):
            # q -> eq = exp(-q) in [D, n]; sigmoid(q)*num/den is computed as
            # num / (den * (1+eq)), so no sigmoid table set is ever needed.
            u = tpool.tile([P, DCH, NSEQ], bf16, tag="u", name="u")
            for mo in range(DCH):
                qps = psp.tile([P, NSEQ], f32, tag="ps", name=f"qps{lsi}_{mo}")
                for ko in range(DCH):
                    nc.tensor.matmul(
                        qps,
                        WqB[:, lsi * DCH + ko, mo * P : (mo + 1) * P],
                        xTb[:, ko, :],
                        start=(ko == 0), stop=(ko == DCH - 1),
                    )
                nc.scalar.activation(u[:, mo, :], qps, AF.Exp, scale=-1.0)
            return u

        def kv_stage(lsi, yTb):
            # k|v packed per seq-chunk, in [m, D]
            ek = tpool.tile([P, SCH, D], bf16, tag="ek", name="ek")
            ekv = tpool.tile([P, SCH, D], bf16, tag="ekv", name="ekv")
            for sc in range(SCH):
                kvps = psp.tile([P, NSEQ], f32, tag="ps", name=f"kvps{lsi}_{sc}")
                for ko in range(DCH):
                    nc.tensor.matmul(
                        kvps,
                        yTb[:, ko, sc * P : (sc + 1) * P],
                        WkvB[:, lsi * DCH + ko, :],
                        start=(ko == 0), stop=(ko == DCH - 1),
                    )
                with tc.high_priority(offset=16):
                    nc.scalar.activation(ek[:, sc, :], kvps[:, 0:D], AF.Exp)
                    nc.vector.tensor_mul(ekv[:, sc, :], kvps[:, D : 2 * D], ek[:, sc, :])
            return ek, ekv

        def aft_stage(lsi, u, ek, ekv, E, xT):
            # AFT: numT/denT [D, n] = (ekv|ek).T @ E^T, then combine + residual
            x1 = tpool.tile([P, DCH, NSEQ], f32, tag="x1", name="x1")
            x1sum = spool.tile([P, DCH], f32, tag="xsum", name="x1sum")
            for do in range(DCH):
                # den first: the (1+eq)*den + reciprocal chain runs on DVE
                # while the num matmuls stream on PE
                dps = psp.tile([P, NSEQ], f32, tag="ps", name=f"dps{lsi}_{do}")
                for sc in range(SCH):
                    nc.tensor.matmul(
                        dps, ek[:, sc, do * P : (do + 1) * P], E[:, sc, :],
                        start=(sc == 0), stop=(sc == SCH - 1),
                    )
                dd = ttp.tile([P, NSEQ], f32, tag="tt", name="dd")
                # dd = (eq + 1) * den  -- folds the sigmoid denominator in
                with tc.high_priority(offset=16):
                    nc.vector.scalar_tensor_tensor(dd, u[:, do, :], 1.0, dps, OP.add, OP.mult)
                rdd = ttp.tile([P, NSEQ], f32, tag="tt", name="rdd")
                nc.vector.reciprocal_approx_fast(rdd, dd)
                nps = psp.tile([P, NSEQ], f32, tag="ps", name=f"nps{lsi}_{do}")
                for sc in range(SCH):
                    nc.tensor.matmul(
                        nps, ekv[:, sc, do * P : (do + 1) * P], E[:, sc, :],
                        start=(sc == 0), stop=(sc == SCH - 1),
                    )
                t = ttp.tile([P, NSEQ], f32, tag="tt", name="t")
                with tc.high_priority(offset=16):
                    nc.vector.tensor_mul(t, nps, rdd)
                nc.vector.scalar_tensor_tensor(
                    x1[:, do, :], t, 0.0, xT[:, do, :], OP.add, OP.add,
                    accum_out=x1sum[:, do : do + 1],
                )
            return x1, x1sum

        def in1_stage(lsi, x1, x1sum):
            # h1 is kept only in bf16: it feeds the ff1 matmuls directly and
            # the ff2 residual add (bf16 rounding there is within budget)
            h1b = tpool.tile([P, DCH, NSEQ], bf16, tag="h1b", name="h1b")
            in_norm(x1, x1sum, g1a, be1a, unit_g1, zero_be1, lsi, None, h1b)
            return h1b

        def ff1_stage(lsi, h1b):
            ff1b = tpool.tile([P, FCH, NSEQ], bf16, tag="ff1b", name="ff1b")
            for fo in range(FCH):
                fps = psp.tile([P, NSEQ], f32, tag="ps", name=f"fps{lsi}_{fo}")
                for ko in range(DCH):
                    nc.tensor.matmul(
                        fps,
                        W1B[:, lsi * DCH + ko, fo * P : (fo + 1) * P],
                        h1b[:, ko, :],
                        start=(ko == 0), stop=(ko == DCH - 1),
                    )
                if zero_b1:
                    nc.scalar.activation(ff1b[:, fo, :], fps, AF.Relu, bias=0.0)
                else:
                    nc.scalar.activation(
                        ff1b[:, fo, :], fps, AF.Relu,
                        bias=b1a[:, lsi * FCH + fo : lsi * FCH + fo + 1],
                    )
            return ff1b

        def ff2_stage(lsi, ff1b, h1b):
            x2 = tpool.tile([P, DCH, NSEQ], f32, tag="x1", name="x2")
            x2sum = spool.tile([P, DCH], f32, tag="xsum", name="x2sum")
            for do in range(DCH):
                f2ps = psp.tile([P, NSEQ], f32, tag="ps", name=f"f2ps{lsi}_{do}")
                for ko in range(FCH):
                    nc.tensor.matmul(
                        f2ps,
                        W2B[:, lsi * FCH + ko, do * P : (do + 1) * P],
                        ff1b[:, ko, :],
                        start=(ko == 0), stop=(ko == FCH - 1),
                    )
                nc.vector.scalar_tensor_tensor(
                    x2[:, do, :], f2ps, 0.0, h1b[:, do, :], OP.add, OP.add,
                    accum_out=x2sum[:, do : do + 1],
                )
            return x2, x2sum

        def in2_stage(lsi, x2, x2sum, last):
            s = lsi % 2
            nx = strm.tile([P, DCH, NSEQ], f32, tag=f"x{s}", name=f"x{s}")
            nxb = None
            if not last:
                nxb = strm.tile([P, DCH, NSEQ], bf16, tag=f"xb{s}", name=f"xb{s}")
            in_norm(x2, x2sum, g2a, be2a, unit_g2, zero_be2, lsi, nx, nxb)
            return nx, nxb

        def enc_pair(l, xs, Er, Ec, last):
            # Interleave the two independent sides of a layer, with the col
            # side staggered ~1.5 stages behind the row side so the two
            # serial norm chains never coincide -- one side's matmuls keep
            # PE fed while the other side's stats/apply chain runs.
            lsr, lsc = l * 2, l * 2 + 1
            (xr, xrb), (xc, xcb) = xs[0], xs[1]
            ur = q_stage(lsr, xrb)
            ekr, ekvr = kv_stage(lsr, xcb)
            uc = q_stage(lsc, xcb)
            x1r, x1sr = aft_stage(lsr, ur, ekr, ekvr, Er, xr)
            ekc, ekvc = kv_stage(lsc, xrb)
            h1br = in1_stage(lsr, x1r, x1sr)
            x1c, x1sc = aft_stage(lsc, uc, ekc, ekvc, Ec, xc)
            f1r = ff1_stage(lsr, h1br)
            h1bc = in1_stage(lsc, x1c, x1sc)
            x2r, x2sr = ff2_stage(lsr, f1r, h1br)
            f1c = ff1_stage(lsc, h1bc)
            nr = in2_stage(lsr, x2r, x2sr, last)
            x2c, x2sc = ff2_stage(lsc, f1c, h1bc)
            ncl = in2_stage(lsc, x2c, x2sc, last)
            return nr, ncl

        for b in range(bloc):
            cm = cmp_.tile([P, SCH, NSEQ], f32, tag="cm", name="cm")
            nc.sync.dma_start(cm, cost_d[b].rearrange("(no ni) m -> ni no m", ni=P))

            def get_Ec(scale, b=b):
                Ec = epool.tile([P, SCH, NSEQ], bf16, tag="Ec", name="Ec")
                for no in range(SCH):
                    nc.scalar.activation(Ec[:, no, :], cm[:, no, :], AF.Exp, scale=scale)
                return Ec

            def get_Er(scale, b=b):
                Er = epool.tile([P, SCH, NSEQ], bf16, tag="Er", name="Er")
                for mo in range(SCH):
                    pt = psp.tile([P, NSEQ], f32, tag="ps", name=f"ept{b}_{mo}")
                    for no in range(SCH):
                        nc.tensor.transpose(
                            pt[:, no * P : (no + 1) * P],
                            cm[:, no, mo * P : (mo + 1) * P],
                            ident,
                        )
                    nc.scalar.activation(Er[:, mo, :], pt, AF.Exp, scale=scale)
                return Er

            xs = {}
            for s, src in ((0, row_d), (1, col_d)):
                xnd = stg.tile([P, SCH, D], f32, tag="xnd", name="xnd")
                nc.sync.dma_start(xnd, src[b].rearrange("(no ni) d -> ni no d", ni=P))
                # dedicated tags for the item-initial stream tiles so the
                # next item's input staging never rotates against this
                # item's per-layer stream tiles (cross-item overlap)
                xT = strm.tile([P, DCH, NSEQ], f32, tag=f"xi{s}", name=f"xi{s}")
                xTb = strm.tile([P, DCH, NSEQ], bf16, tag=f"xbi{s}", name=f"xbi{s}")
                for do in range(DCH):
                    pt = psp.tile([P, NSEQ], f32, tag="ps", name=f"xpt{b}_{s}_{do}")
                    for no in range(SCH):
                        nc.tensor.transpose(
                            pt[:, no * P : (no + 1) * P],
                            xnd[:, no, do * P : (do + 1) * P],
                            ident,
                        )
                    nc.vector.tensor_copy(xT[:, do, :], pt)
                    nc.scalar.copy(xTb[:, do, :], pt)
                xs[s] = (xT, xTb)

            er_scale = ec_scale = None
            Er = Ec = None
            for l in range(L):
                if scales_r[l] != er_scale:
                    Er = get_Er(scales_r[l])
                    er_scale = scales_r[l]
                if scales_c[l] != ec_scale:
                    Ec = get_Ec(scales_c[l])
                    ec_scale = scales_c[l]
                last = l == L - 1
                nr, ncl = enc_pair(l, xs, Er, Ec, last)
                xs[0], xs[1] = nr, ncl

            for s in (0, 1):
                nx = xs[s][0]
                ond = stg.tile([P, SCH, D], f32, tag="ond", name="ond")
                for no in range(SCH):
                    ops_ = psp.tile([P, D], f32, tag="ps", name=f"ops{b}_{s}_{no}")
                    for do in range(DCH):
                        nc.tensor.transpose(
                            ops_[:, do * P : (do + 1) * P],
                            nx[:, do, no * P : (no + 1) * P],
                            ident,
                        )
                    nc.vector.tensor_copy(ond[:, no, :], ops_)
                nc.sync.dma_start(
                    out_d[s, b].rearrange("(no ni) d -> ni no d", ni=P), ond
                )

    nc.compile()
    return nc


def _get_compiled(scales_r, scales_c, flags):
    from concourse.bass_interp import get_hw_module

    key = (scales_r, scales_c, flags)
    if key not in _CACHE:
        nc = _build(scales_r, scales_c, **dict(flags))
        nc.m = get_hw_module(nc.m)
        _CACHE[key] = nc
    return _CACHE[key]


def kernel(**inputs) -> np.ndarray:
    global LAST_RESULT
    from concourse import bass_utils

    def f32c(x):
        return np.ascontiguousarray(np.asarray(x, dtype=np.float32))

    log_scale = float(np.asarray(inputs["log_scale"]))
    alpha = np.asarray(inputs["alpha"], dtype=np.float64)
    scales_r = tuple(float(-log_scale * alpha[l, 0]) for l in range(L))
    scales_c = tuple(float(-log_scale * alpha[l, 1]) for l in range(L))

    flags = (
        ("unit_g1", bool(np.all(np.asarray(inputs["g1"]) == 1.0))),
        ("zero_be1", bool(np.all(np.asarray(inputs["be1"]) == 0.0))),
        ("unit_g2", bool(np.all(np.asarray(inputs["g2"]) == 1.0))),
        ("zero_be2", bool(np.all(np.asarray(inputs["be2"]) == 0.0))),
        ("zero_b1", bool(np.all(np.asarray(inputs["b1"]) == 0.0))),
    )
    nc = _get_compiled(scales_r, scales_c, flags)

    shard_names = ("row_emb", "col_emb", "cost_mat")
    rep_names = ("Wq", "Wk", "Wv", "g1", "be1", "W1", "b1", "W2", "g2", "be2")
    rep = {k: f32c(inputs[k]) for k in rep_names}
    in_maps = []
    for c in range(NCORES):
        m = dict(rep)
        for k in shard_names:
            m[k] = f32c(np.asarray(inputs[k])[c * BLOC : (c + 1) * BLOC])
        in_maps.append(m)

    res = bass_utils.run_bass_kernel_spmd(nc, in_maps, core_ids=list(range(NCORES)))
    LAST_RESULT = res
    out = np.concatenate([res.results[c]["out"] for c in range(NCORES)], axis=1)
    return out
